# revision 29
# baseline (speedup 1.0000x reference)
"""MoE linear (modality-routed) Trainium2 kernel.

out[n] = x[n] @ W[modality_ids[n]].T + b[modality_ids[n]]

Strategy (data parallel over 8 cores, weight replicated):
- Host: per-core shard of 16384 tokens, stable-sort tokens by expert and pad
  each expert segment to a shared 128-aligned capacity (one SPMD NEFF serves
  all cores; the expert of each 128-token subtile is a compile-time
  constant). x is cast to bf16 (rel tolerance 2e-2; bf16 matmul w/ fp32
  accum lands at ~4e-3) and pre-transposed into a [128, n_subtiles, 512]
  layout so every device DMA is a plain contiguous HWDGE transfer.
- Device per group of up to 4 subtiles (512 tokens): one ~512KB load of x^T,
  4 accumulating bf16 matmuls per subtile (lhsT = x^T chunk stationary,
  rhs = SBUF-resident W^T, fp32 PSUM), DVE bias-add + bf16 downcast, one
  ~512KB store of the y group.
- Host: invert the layout + permutation, upcast to fp32.
"""

import sys

if "/opt/trn_rl_repo" not in sys.path:
    sys.path.insert(0, "/opt/trn_rl_repo")

import ml_dtypes
import numpy as np

import concourse.bass as bass  # noqa: F401
import concourse.tile as tile
from concourse import bacc, mybir
from concourse.bass_utils import run_bass_kernel_spmd

N_CORES = 8
N_TOKENS = 131072
N_SHARD = N_TOKENS // N_CORES  # 16384
D_IN = 512
D_OUT = 512
N_EXPERTS = 3
P = 128
KC = D_IN // P  # 4 contraction chunks
GSUB = 4  # subtiles per group (DMA batch): 512 tokens

BF16 = ml_dtypes.bfloat16
WARMUPS = 140  # PE clock-ramp warmup matmuls

_NC_CACHE = {}


def _groups_of(caps):
    """[(subtile_start, n_subtiles, expert), ...] with n_subtiles <= GSUB."""
    groups = []
    st = 0
    for e, c in enumerate(caps):
        n = c // P
        while n > 0:
            m = min(n, GSUB)
            groups.append((st, m, e))
            st += m
            n -= m
    return groups


def build_nc(caps, num_devices=N_CORES):
    """Build + compile the SPMD Bass kernel for given per-expert capacities
    (each a multiple of P)."""
    key = (tuple(caps), num_devices)
    if key in _NC_CACHE:
        return _NC_CACHE[key]
    npad = sum(caps)
    nst = npad // P
    groups = _groups_of(caps)

    nc = bacc.Bacc(
        "TRN2", target_bir_lowering=False, debug=False, num_devices=num_devices
    )
    f32 = mybir.dt.float32
    bf16 = mybir.dt.bfloat16

    # x^T, sorted by expert: xt[p, st, kc*P + t] = x_sorted[st*P + t, kc*P + p]
    xt = nc.dram_tensor("xt", [P, nst, KC * P], bf16, kind="ExternalInput").ap()
    # W^T blocks: wt[p, (e*KC+kc)*D_OUT + o] = W[e*D_OUT + o, kc*P + p]
    wt = nc.dram_tensor("wt", [P, N_EXPERTS * KC * D_OUT], bf16, kind="ExternalInput").ap()
    # bias broadcast across partitions: bb[p, e*D_OUT + o] = b[e*D_OUT + o]
    bb = nc.dram_tensor("bias_bc", [P, N_EXPERTS * D_OUT], bf16, kind="ExternalInput").ap()
    # y[p, st, o] = y_sorted[st*P + p, o]
    y = nc.dram_tensor("y", [P, nst, D_OUT], bf16, kind="ExternalOutput").ap()

    with tile.TileContext(nc) as tc:
        with (
            tc.tile_pool(name="const", bufs=1) as cpool,
            tc.tile_pool(name="xg", bufs=8) as xg_pool,
            tc.tile_pool(name="outp", bufs=6) as out_pool,
            tc.tile_pool(name="pmm", bufs=7, space="PSUM") as pmm_pool,
            tc.tile_pool(name="wps", bufs=1, space="PSUM") as wps_pool,
        ):
            w_sb = cpool.tile([P, N_EXPERTS * KC * D_OUT], bf16)
            bias_sb = cpool.tile([P, N_EXPERTS * D_OUT], bf16)

            # PE warmup: tiny matmuls fill the otherwise-idle DMA startup
            # window and get the PE clock ramp to full speed before the first
            # real matmul is issued. They read a w_sb slice whose DMA arrives
            # much later (expert 2's last chunk), so the only dependency is a
            # harmless write-after-read on that late DMA.
            warm = w_sb[:, N_EXPERTS * KC * D_OUT - 32 :]
            wps = wps_pool.tile([32, 32], f32)
            for _ in range(WARMUPS):
                nc.tensor.matmul(
                    wps[:], lhsT=warm, rhs=warm, start=True, stop=True
                )

            ngroups = len(groups)
            e_first = groups[0][2]
            # Later experts' weights+bias stream in as small per-kc pieces
            # spread over earlier phases, amortized into per-group DMA slack.
            first_gi_of_e = {}
            for gi, (_, _, e) in enumerate(groups):
                first_gi_of_e.setdefault(e, gi)
            const_sched = {}  # gi -> [("w"|"b", (lo, hi) column range)]
            for e in range(N_EXPERTS):
                if caps[e] == 0 or e == e_first:
                    continue
                need_by = max(1, first_gi_of_e[e])
                start = max(1, need_by - 7)
                for kc in range(KC):
                    const_sched.setdefault(min(start + kc, need_by - 1), []).append(
                        ("w", ((e * KC + kc) * D_OUT, (e * KC + kc + 1) * D_OUT))
                    )
                const_sched.setdefault(min(start + KC, need_by - 1), []).append(
                    ("b", (e * D_OUT, (e + 1) * D_OUT))
                )
            for gi, (st0, m, e) in enumerate(groups):
                xg = xg_pool.tile([P, m * KC * P], bf16)
                nc.sync.dma_start(out=xg[:], in_=xt[:, st0 : st0 + m, :])
                if gi == 0:
                    # First group's weights arrive per-kc chunk right behind
                    # its x tile; bias + later experts stream in behind
                    # compute via const_sched.
                    for kc in range(KC):
                        lo = (e_first * KC + kc) * D_OUT
                        nc.sync.dma_start(
                            out=w_sb[:, lo : lo + D_OUT], in_=wt[:, lo : lo + D_OUT]
                        )
                    nc.sync.dma_start(
                        out=bias_sb[:, e_first * D_OUT : (e_first + 1) * D_OUT],
                        in_=bb[:, e_first * D_OUT : (e_first + 1) * D_OUT],
                    )
                osb = out_pool.tile([P, m * D_OUT], bf16)
                last_group = gi == ngroups - 1
                for sub in range(m):
                    pmm = pmm_pool.tile([P, D_OUT], f32)
                    for kc in range(KC):
                        nc.tensor.matmul(
                            pmm[:],
                            lhsT=xg[
                                :, sub * KC * P + kc * P : sub * KC * P + (kc + 1) * P
                            ],
                            rhs=w_sb[
                                :, (e * KC + kc) * D_OUT : (e * KC + kc + 1) * D_OUT
                            ],
                            start=(kc == 0),
                            stop=(kc == KC - 1),
                        )
                    nc.vector.tensor_add(
                        out=osb[:, sub * D_OUT : (sub + 1) * D_OUT],
                        in0=pmm[:],
                        in1=bias_sb[:, e * D_OUT : (e + 1) * D_OUT],
                    )
                    if last_group:
                        # Per-subtile stores so earlier stores overlap the
                        # remaining matmuls and the final transfer is small.
                        nc.sync.dma_start(
                            out=y[:, st0 + sub, :],
                            in_=osb[:, sub * D_OUT : (sub + 1) * D_OUT],
                        )
                if not last_group:
                    nc.sync.dma_start(out=y[:, st0 : st0 + m, :], in_=osb[:])
                for kind, (lo, hi) in const_sched.get(gi, ()):
                    if kind == "w":
                        nc.sync.dma_start(out=w_sb[:, lo:hi], in_=wt[:, lo:hi])
                    else:
                        nc.sync.dma_start(out=bias_sb[:, lo:hi], in_=bb[:, lo:hi])

    nc.compile()
    _NC_CACHE[key] = nc
    return nc


def prepare(inputs):
    """Host-side prep: returns (nc, in_maps, posts) where posts[c] is
    (order, seg) needed to unscramble core c's output."""
    x = np.asarray(inputs["x"], dtype=np.float32)
    ids = np.asarray(inputs["modality_ids"]).astype(np.int64)
    weight = np.asarray(inputs["weight"], dtype=np.float32)
    b = np.asarray(inputs["bias"], dtype=np.float32)

    # W^T blocks in bf16: wt_dev[p, (e*KC+kc)*D_OUT + o] = W[e*D_OUT+o, kc*P+p]
    w3 = weight.reshape(N_EXPERTS, D_OUT, KC, P)  # [e, o, kc, p]
    wt_dev = np.ascontiguousarray(
        w3.transpose(3, 0, 2, 1).reshape(P, N_EXPERTS * KC * D_OUT)
    ).astype(BF16)
    bias_bc = np.ascontiguousarray(
        np.broadcast_to(b[None, :], (P, N_EXPERTS * D_OUT))
    ).astype(BF16)

    # Balanced sharding: tokens of each expert are dealt near-evenly across the
    # 8 cores (the shard assignment is ours to choose — we un-permute at the
    # end), which minimizes the shared per-expert capacity padding.
    chunks = []  # chunks[e][c] = global token indices of expert e on core c
    for e in range(N_EXPERTS):
        idx_e = np.nonzero(ids == e)[0]
        chunks.append(np.array_split(idx_e, N_CORES))
    caps = [
        int(-(-max(len(ch) for ch in chunks[e]) // P) * P)
        for e in range(N_EXPERTS)
    ]
    npad = sum(caps)
    nst = npad // P

    nc = build_nc(caps)
    in_maps = []
    posts = []
    xb = x.astype(BF16)
    for c in range(N_CORES):
        xs = np.zeros((npad, D_IN), dtype=BF16)
        base = 0
        seg = []  # (global_indices, base) per expert
        for e in range(N_EXPERTS):
            gidx = chunks[e][c]
            cc = len(gidx)
            xs[base : base + cc] = xb[gidx]
            seg.append((gidx, base))
            base += caps[e]
        # xt_dev[p, st, kc*P + t] = xs[st*P + t, kc*P + p]
        xt_dev = np.ascontiguousarray(
            xs.reshape(nst, P, KC, P).transpose(3, 0, 2, 1).reshape(P, nst, KC * P)
        )
        in_maps.append({"xt": xt_dev, "wt": wt_dev, "bias_bc": bias_bc})
        posts.append(seg)
    return nc, in_maps, posts


def run(inputs, trace=False):
    """Returns (out, BassKernelResults)."""
    nc, in_maps, posts = prepare(inputs)
    res = run_bass_kernel_spmd(nc, in_maps, list(range(N_CORES)), trace=trace)
    out = np.empty((N_TOKENS, D_OUT), dtype=np.float32)
    for c in range(N_CORES):
        y_dev = np.asarray(res.results[c]["y"])  # [P, nst, D_OUT] bf16
        nst = y_dev.shape[1]
        # y_sorted[st*P + p, o] = y_dev[p, st, o]
        y_sorted = (
            y_dev.transpose(1, 0, 2).reshape(nst * P, D_OUT).astype(np.float32)
        )
        for gidx, base in posts[c]:
            out[gidx] = y_sorted[base : base + len(gidx)]
    return out, res


def kernel(**inputs):
    out, _ = run(inputs, trace=False)
    return out
